# revision 23
# baseline (speedup 1.0000x reference)
"""Bass/Trainium2 kernel for nn_BasicBlock_73933567033945 (CDConv / gnn_message_passing).

Strategy: graph is a fixed +-8 sequence window inside each of 4 chains of
L=2048 nodes (verified against the src/dst inputs at runtime). Shard the
8192 nodes across 8 NeuronCores (1024 nodes each) with an 8-node halo.

Per core (banded-weights formulation):
  conv[n, o] = sum_{m, c} Band_c[m, n] * G[m, c*32+o]
where G = h @ Wk-fold (per-tile slab of 128 halo rows, fp8) and
Band_c[m, n] = kern(edge n<-m) is a banded node x node matrix per kernel
channel c. The per-edge kernel kern[m, (j, c)] (j = n - m + 16) comes
from one PE matmul against a block-diagonal WS (119 features; the bias
rides the always-zero self-edge dist feature), lrelu'd, masked, fp8.
Band materialization: one fat diagonal scatter DMA (row m -> DRAM at
stride 3096 = 3072 + 24 skew, 408B rows) plus one fat contiguous read
back ([128 x 2688B] rows) into the (n, c)-interleaved band; a single
uint32 DVE pass regroups it into quad-interleaved planes. The message
passing is then 12 PSUM-accumulated fp8 DoubleRow matmuls per 112-node
tile (dual-row contraction over c-pairs), replacing the per-edge DVE
products and PE shift-matmul scatter of the direct implementation.
Geometry for the edge features is computed batched (2 tiles per pass)
to keep the cross-engine dependency chains off the critical path.
"""
import numpy as np

B, L, C = 4, 2048, 128
N = B * L
W = 32
KC = 24
SEQ_L = 11
R = 12.0
WIN = 8
NEG_IN = 0.1
NEG_K = 0.2
NCORES = 8
NPC = N // NCORES          # 1024 nodes per core
TS = 112                   # output nodes per tile
NT = 10                    # tiles per core (9*112 + 16)
HALO = 1152                # padded halo rows per core (needs 1136)
K17 = 2 * WIN + 1          # 17 window offsets
S_HALF = SEQ_L // 2
GW = W * KC                # 768 = width of a G slab
BW = KC * TS               # 2688 = band width
NF = K17 * 7               # 119 geometry features (bias in (j=8, d=6))
KW = K17 * KC              # 408 kern cols (j*24 + c)
DROW = 3096                # dram band scatter row stride (3072 + 24 skew)

_PROG = {}


def _sidx_j(j):
    # seq-kernel index for window offset (8 - j)
    return int(np.clip(8 - j, -S_HALF, S_HALF)) + S_HALF


def _build_program():
    import concourse.tile as tile
    from concourse import mybir, bacc
    from concourse.ap import AP
    from contextlib import ExitStack

    f32 = mybir.dt.float32
    bf16 = mybir.dt.bfloat16
    fp8 = mybir.dt.float8e4
    u32 = mybir.dt.uint32
    AF = mybir.ActivationFunctionType
    OP = mybir.AluOpType
    AX = mybir.AxisListType
    DR = mybir.MatmulPerfMode.DoubleRow

    nc = bacc.Bacc("TRN2", target_bir_lowering=False, debug=False)

    def din(name, shape, dt=f32):
        return nc.dram_tensor(name, shape, dt, kind="ExternalInput").ap()

    xc_slot = din("xc_slot", [128, NT * C])
    gpack_f = din("gpack_f", [128, 540])
    gpack_bf = din("gpack_bf", [128, 1620], bf16)
    xT_slot = din("xT_slot", [128, HALO], bf16)
    wpack_bf = din("wpack_bf", [128, 1634], bf16)
    wpack_f = din("wpack_f", [128, 108])
    y = nc.dram_tensor("y", [NPC, C], f32, kind="ExternalOutput").ap()

    with tile.TileContext(nc) as tc, ExitStack() as ctx:
        pers = ctx.enter_context(tc.tile_pool(name="pers", bufs=1))
        dpool = ctx.enter_context(tc.tile_pool(name="dpool", bufs=1,
                                               space="DRAM"))

        def load(ap_in, shape, tag, dt=f32, eng=None):
            t = pers.tile(shape, dt, tag=tag)
            (eng or nc.sync).dma_start(t[:], ap_in)
            return t

        gf = load(gpack_f, [128, 540], "gf")
        gbf = load(gpack_bf, [128, 1620], "gbf", bf16)
        xT_sb = load(xT_slot, [128, HALO], "xT", bf16, eng=nc.scalar)
        wbf = load(wpack_bf, [128, 1634], "wbf", bf16, eng=nc.scalar)
        wf = load(wpack_f, [128, 108], "wf", eng=nc.scalar)
        xc_all = load(xc_slot, [128, NT * C], "xc_all", eng=nc.scalar)
        pos_all = gf[:, 0:30]
        pgat = gf[:, 30:540]
        ori_all = gbf[:, 0:90]
        ogat = gbf[:, 90:1620]
        id_b = wbf[:, 0:128]
        mask_sb = wbf[:, 128:298]
        ws_b = wbf[0:NF, 298:706]
        wkoc_b = wbf[0:W, 706:1474]
        w_out_b = wbf[0:W, 1474:1602]
        w_in_b = wbf[:, 1602:1634]
        w5r_sb = wf[:, 0:72]
        b5r_sb = wf[:, 72:96]
        ncl_sb = wf[:, 96:106]
        a1_sb = wf[:, 106:107]
        a2_sb = wf[:, 107:108]

        # DRAM band buffers, zero-initialized once
        zraw = pers.tile([128, DROW], fp8, tag="zraw")
        nc.gpsimd.memset(zraw[:], 0.0)
        dbands = []
        for i in range(2):
            db = dpool.tile([128 * DROW], fp8, tag=f"dband{i}",
                            name=f"dband{i}")
            nc.sync.dma_start(
                AP(db.tensor, db.offset, [[DROW, 128], [1, DROW]]), zraw[:])
            dbands.append(db)
        bands = [pers.tile([128, BW], fp8, tag=f"band{i}", name=f"band{i}")
                 for i in range(3)]
        bq_t = [pers.tile([128, BW], fp8, tag=f"bq{i}", name=f"bq{i}")
                for i in range(3)]

        hT = pers.tile([W, HALO], bf16, tag="hT")
        da_bt = [pers.tile([128, 256], bf16, tag=f"da{i}", name=f"da{i}")
                 for i in range(5)]
        kself_all = pers.tile([128, NT * KC], bf16, tag="kself_all")

        # ---------------- Phase A: hT = lrelu(W_in^T @ lrelu(x)^T) ---------
        with tc.tile_pool(name="pA", bufs=2) as pA, \
             tc.tile_pool(name="pAp", bufs=2, space="PSUM") as pAp:
            xlT = pA.tile([128, HALO], bf16, tag="xlT")
            for s in range(3):
                sl = slice(384 * s, 384 * (s + 1))
                nc.scalar.activation(xlT[:, sl], xT_sb[:, sl], AF.Prelu,
                                     bias=0.0, scale=1.0, alpha=a1_sb[:, 0:1])
                hp = pAp.tile([W, 384], f32, tag="hp")
                nc.tensor.matmul(hp[:], w_in_b[:], xlT[:, sl],
                                 start=True, stop=True)
                nc.scalar.activation(hT[:, sl], hp[:], AF.Prelu,
                                     bias=0.0, scale=1.0, alpha=a1_sb[0:W, 0:1])

        wrk = ctx.enter_context(tc.tile_pool(name="wrk", bufs=3))
        kpool = ctx.enter_context(tc.tile_pool(name="kp", bufs=3))
        gpool = ctx.enter_context(tc.tile_pool(name="gp", bufs=4))
        psK = ctx.enter_context(tc.tile_pool(name="psK", bufs=2, space="PSUM"))
        psT = ctx.enter_context(tc.tile_pool(name="psT", bufs=1, space="PSUM"))
        psG = ctx.enter_context(tc.tile_pool(name="psG", bufs=1, space="PSUM"))
        psC = ctx.enter_context(tc.tile_pool(name="psC", bufs=2, space="PSUM"))
        psO = ctx.enter_context(tc.tile_pool(name="psO", bufs=1, space="PSUM"))

        P = 128

        def geo_batch(bi):
            """Edge geometry for tiles (2*bi, 2*bi+1) -> da_bt[bi]; also
            kself for those tiles."""
            t0, nb = 2 * bi, 2
            TK = nb * K17
            da = da_bt[bi]
            dav = da[:, 0:238].rearrange("p (tj d) -> p tj d", d=7)
            pos_nb = pgat[:, 51 * t0:51 * (t0 + nb)] \
                .rearrange("p (tb j d) -> p tb j d", j=K17, d=3)
            ori_nb = ogat[:, 153 * t0:153 * (t0 + nb)] \
                .rearrange("p (tb j ab) -> p tb j ab", j=K17, ab=9)
            poN_pos = pos_all[:, 3 * t0:3 * (t0 + nb)] \
                .rearrange("p (tb d) -> p tb d", d=3).unsqueeze(2) \
                .broadcast_to([P, nb, K17, 3])
            poN_ori = ori_all[:, 9 * t0:9 * (t0 + nb)] \
                .rearrange("p (tb ab) -> p tb ab", ab=9).unsqueeze(2) \
                .broadcast_to([P, nb, K17, 9])

            D = wrk.tile([P, TK * 3], f32, tag="D")
            Dv = D[:].rearrange("p (tj a) -> p tj a", a=3)
            nc.vector.tensor_sub(D[:].rearrange("p (tb j d) -> p tb j d",
                                                j=K17, d=3),
                                 poN_pos, pos_nb)
            sq = wrk.tile([P, TK * 3], f32, tag="sq")
            nc.vector.tensor_mul(sq[:], D[:], D[:])
            d2 = wrk.tile([P, TK], f32, tag="d2")
            nc.vector.tensor_reduce(d2[:],
                                    sq[:].rearrange("p (tj a) -> p tj a", a=3),
                                    axis=AX.X, op=OP.add)
            nc.scalar.activation(dav[:, :, 6], d2[:], AF.Sqrt, bias=0.0,
                                 scale=1.0 / (R * R))
            dist = wrk.tile([P, TK], f32, tag="dist")
            nc.scalar.activation(dist[:], d2[:], AF.Sqrt, bias=0.0, scale=1.0)
            rec = wrk.tile([P, TK], f32, tag="rec")
            nc.vector.tensor_scalar_add(dist[:], dist[:], 1e-9)
            nc.vector.reciprocal(rec[:], dist[:])
            dirn = wrk.tile([P, TK * 3], bf16, tag="dirn")
            nc.vector.tensor_mul(dirn[:].rearrange("p (tj a) -> p tj a", a=3),
                                 Dv,
                                 rec[:].unsqueeze(-1).broadcast_to([P, TK, 3]))
            lm = wrk.tile([P, TK * 9], bf16, tag="lm")
            nc.gpsimd.tensor_mul(
                lm[:].rearrange("p (tj a b) -> p tj a b", a=3, b=3),
                ori_nb.rearrange("p tb j (a b) -> p (tb j) a b", b=3),
                dirn[:].rearrange("p (tj b) -> p tj b", b=3).unsqueeze(2)
                       .broadcast_to([P, TK, 3, 3]))
            with nc.allow_low_precision(reason="3-term bf16 geometry sums"):
                nc.vector.tensor_reduce(
                    dav[:, :, 0:3],
                    lm[:].rearrange("p (tj a b) -> p tj a b", a=3, b=3),
                    axis=AX.X, op=OP.add)
            ofm = wrk.tile([P, TK * 9], bf16, tag="ofm")
            nc.gpsimd.tensor_mul(
                ofm[:].rearrange("p (tb j ab) -> p tb j ab", j=K17, ab=9),
                ori_nb, poN_ori)
            with nc.allow_low_precision(reason="3-term bf16 geometry sums"):
                nc.vector.tensor_reduce(
                    dav[:, :, 3:6],
                    ofm[:].rearrange("p (tj a b) -> p tj a b", a=3, b=3),
                    axis=AX.X, op=OP.add)
            # bias carrier: self-edge dist feature (j=8, d=6) := 1
            nc.vector.memset(
                da[:, 0:238].rearrange("p (tb j d) -> p tb j d", j=K17, d=7)
                     [:, :, 8, 6], 1.0)

            # ---- kself for these tiles (folded self-loops), on gpsimd ----
            if bi not in (0, 4):
                return
            sqo = wrk.tile([P, nb * 9], f32, tag="sqo")
            ov = ori_all[:, 9 * t0:9 * (t0 + nb)]
            nc.gpsimd.tensor_mul(sqo[:], ov, ov)
            rn = wrk.tile([P, nb * 3], f32, tag="rn")
            nc.vector.tensor_reduce(
                rn[:], sqo[:].rearrange("p (tb a b) -> p tb a b", a=3, b=3),
                axis=AX.X, op=OP.add)
            rnv = rn[:].rearrange("p (tb a) -> p tb a", a=3)
            ks = wrk.tile([P, nb * KC], f32, tag="ks")
            ksv = ks[:].rearrange("p (tb c) -> p tb c", c=KC)
            tmpk = wrk.tile([P, nb * KC], f32, tag="tmpk")
            tmv = tmpk[:].rearrange("p (tb c) -> p tb c", c=KC)
            nc.gpsimd.tensor_mul(
                ksv, rnv[:, :, 0:1].broadcast_to([P, nb, KC]),
                w5r_sb[:, 0:KC].unsqueeze(1).broadcast_to([P, nb, KC]))
            nc.gpsimd.tensor_mul(
                tmv, rnv[:, :, 1:2].broadcast_to([P, nb, KC]),
                w5r_sb[:, KC:2 * KC].unsqueeze(1).broadcast_to([P, nb, KC]))
            nc.gpsimd.tensor_add(ks[:], ks[:], tmpk[:])
            nc.gpsimd.tensor_mul(
                tmv, rnv[:, :, 2:3].broadcast_to([P, nb, KC]),
                w5r_sb[:, 2 * KC:3 * KC].unsqueeze(1).broadcast_to([P, nb, KC]))
            nc.gpsimd.tensor_add(ks[:], ks[:], tmpk[:])
            nc.gpsimd.tensor_add(
                ksv, ksv, b5r_sb[:].unsqueeze(1).broadcast_to([P, nb, KC]))
            nc.scalar.activation(ks[:], ks[:], AF.Prelu, bias=0.0,
                                 scale=1.0, alpha=a2_sb[:, 0:1])
            nc.gpsimd.tensor_mul(
                kself_all[:, KC * t0:KC * (t0 + nb)]
                    .rearrange("p (tb c) -> p tb c", c=KC),
                ksv,
                ncl_sb[:, t0:t0 + nb].unsqueeze(-1).broadcast_to([P, nb, KC]))

        def stage_a(t):
            # ---- kern[m, (j, c)] = mask * lrelu(da @ WS) ------------------
            da = da_bt[t // 2][:, NF * (t % 2):NF * (t % 2 + 1)]
            dT_ps = psT.tile([NF, 128], bf16, tag="dT")
            nc.tensor.matmul(dT_ps[:], da, id_b, is_transpose=True,
                             start=True, stop=True, skip_group_check=True)
            dT = wrk.tile([NF, 128], bf16, tag="dT_sb")
            nc.scalar.copy(dT[:], dT_ps[:])
            pre_ps = psK.tile([P, KW], f32, tag="pre")
            nc.tensor.matmul(pre_ps[:], dT[:], ws_b[:], start=True,
                             stop=True, skip_group_check=True)
            k8 = kpool.tile([P, KW], fp8, tag="k8")
            nc.scalar.activation(k8[:], pre_ps[:], AF.Prelu,
                                 bias=0.0, scale=1.0, alpha=a2_sb[:, 0:1])
            if t in (0, NT - 1):
                # chain-boundary tile: zero invalid edges, add folded
                # self-loop kernels (interior tiles: mask==1, ncl==0)
                kv = k8[:].rearrange("p (j c) -> p j c", c=KC)
                nc.gpsimd.tensor_mul(
                    kv, kv,
                    mask_sb[:, K17 * t:K17 * (t + 1)].unsqueeze(-1)
                           .broadcast_to([P, K17, KC]))
                nc.gpsimd.tensor_add(kv[:, 8:9, :], kv[:, 8:9, :],
                                     kself_all[:, KC * t:KC * (t + 1)]
                                     .unsqueeze(1))

            # ---- G slab (fp8, (c, o) layout) ------------------------------
            gp_ps = psG.tile([128, GW], f32, tag="gp")
            nc.tensor.matmul(gp_ps[:, 0:512], hT[:, TS * t:TS * t + 128],
                             wkoc_b[:, 0:512], start=True, stop=True,
                             skip_group_check=True)
            nc.tensor.matmul(gp_ps[:, 512:GW], hT[:, TS * t:TS * t + 128],
                             wkoc_b[:, 512:GW], start=True, stop=True,
                             skip_group_check=True)
            G = gpool.tile([128, GW], fp8, tag="G")
            nc.scalar.copy(G[:, 0:384], gp_ps[:, 0:384])
            nc.vector.tensor_copy(G[:, 384:GW], gp_ps[:, 384:GW])

            # ---- fat scatter + fat read -----------------------------------
            db, bd = dbands[t % 2], bands[t % 3]
            sdst = AP(db.tensor, db.offset, [[DROW, 128], [1, KW]])
            nc.sync.dma_start(sdst, k8[:])
            rsrc = AP(db.tensor, db.offset + 384, [[3072, 128], [1, BW]])
            nc.sync.dma_start(bd[:], rsrc)
            return G, bd

        def rearr(t, bd):
            # (n, c)-interleaved -> quad-interleaved planes, as uint32
            bq = bq_t[t % 3]
            src = bd[:].bitcast(u32).rearrange("p (n q) -> p q n", q=6)
            dst = bq[:].bitcast(u32).rearrange("p (q n) -> p q n", n=TS)
            nc.vector.tensor_copy(dst, src)
            return bq

        def stage_b(t, G, bq):
            conv_ps = psC.tile([W, TS], f32, tag="conv")
            for q in range(KC // 2):
                q4, h = q // 2, q % 2
                lhsT = G[:, 64 * q:64 * (q + 1)] \
                    .rearrange("p (b o) -> p b o", b=2)
                rhs = bq[:, 448 * q4:448 * (q4 + 1)] \
                    .rearrange("p (n b4) -> p n b4", b4=4)[:, :, 2 * h:2 * h + 2] \
                    .rearrange("p n b -> p b n")
                nc.tensor.matmul(conv_ps[:], lhsT, rhs, start=(q == 0),
                                 stop=(q == KC // 2 - 1), perf_mode=DR,
                                 skip_group_check=True)
            convLT = wrk.tile([W, TS], bf16, tag="convLT")
            nc.scalar.activation(convLT[:], conv_ps[:], AF.Prelu, bias=0.0,
                                 scale=1.0, alpha=a1_sb[0:W, 0:1])
            out_ps = psO.tile([TS, C], f32, tag="out")
            nc.tensor.matmul(out_ps[:], convLT[:], w_out_b[:],
                             start=True, stop=True, skip_group_check=True)
            out_sb = wrk.tile([TS, C], f32, tag="out_sb")
            nc.vector.tensor_add(out_sb[:], out_ps[:],
                                 xc_all[0:TS, C * t:C * t + C])
            cnt = min(TS, NPC - TS * t)
            nc.sync.dma_start(y[TS * t:TS * t + cnt, :], out_sb[0:cnt, :])

        geo_batch(0)
        pend = {}
        bqp = {}
        for i in range(NT + 2):
            if i >= 2:
                t = i - 2
                stage_b(t, pend.pop(t)[0], bqp.pop(t))
            if 1 <= i <= NT:
                t = i - 1
                bqp[t] = rearr(t, pend[t][1])
            if i < NT:
                if i in (0, 2, 4, 6):
                    geo_batch(i // 2 + 1)
                pend[i] = stage_a(i)

    nc.compile()
    return nc


def _expected_src_dst():
    i = np.arange(N)
    offs = np.arange(-WIN, WIN + 1)
    j = i[:, None] + offs[None, :]
    valid = ((j // L) == (i[:, None] // L)) & (j >= 0) & (j < N)
    j = np.where(valid, j, i[:, None])
    dst = np.repeat(i, offs.size).astype(np.int32)
    src = j.reshape(-1).astype(np.int32)
    return src, dst


def _host_inputs(x, pos, ori, W_in, Ws0, bs0, Wk, W_out):
    import ml_dtypes
    bf = ml_dtypes.bfloat16
    xf = np.ascontiguousarray(x.reshape(N, C), np.float32)
    pos = np.asarray(pos, np.float32)
    ori = np.asarray(ori, np.float32)

    # WS [119, 408]: rows = (j, d<7); row (8,6) carries the shared bias;
    # cols = j*24 + c
    WS = np.zeros((NF, KW), np.float32)
    for j in range(K17):
        s = _sidx_j(j)
        for c in range(KC):
            WS[7 * j:7 * j + 7, KC * j + c] = Ws0[s][:, c]
            WS[7 * 8 + 6, KC * j + c] = bs0[s][c]
    WS[7 * 8 + 6, KC * 8:KC * 9] = bs0[5]   # row overwritten above; re-set
    # wkoc[w, c*32+o] = Wk[c*32+w, o]   ((c, o) layout)
    wkoc = np.ascontiguousarray(
        np.transpose(np.asarray(Wk, np.float32).reshape(KC, W, W),
                     (1, 0, 2)).reshape(W, GW))
    w5r = np.tile(Ws0[5][3:6].reshape(1, 3 * KC), (128, 1)).astype(np.float32)
    b5r = np.tile(bs0[5].reshape(1, KC), (128, 1)).astype(np.float32)
    wf = np.zeros((128, 108), np.float32)
    wf[:, 0:72] = w5r
    wf[:, 72:96] = b5r
    wf[:, 106] = NEG_IN
    wf[:, 107] = NEG_K

    offs = np.arange(-WIN, WIN + 1)
    in_maps = []
    for ci in range(NCORES):
        s0 = ci * NPC
        g = s0 - WIN + np.arange(HALO)
        ok = (g >= 0) & (g < N)
        gi = np.clip(g, 0, N - 1)
        x_pad = np.where(ok[:, None], xf[gi], 0.0).astype(np.float32)
        p_pad = np.where(ok[:, None], pos[gi], 0.0).astype(np.float32)
        o_pad = np.where(ok[:, None], ori[gi], 0.0).astype(np.float32)

        xT_slot = np.ascontiguousarray(x_pad.T).astype(bf)      # [128, HALO]

        jj, pp = np.meshgrid(np.arange(NT), np.arange(128), indexing="ij")
        rows = (TS * jj + pp)            # [NT, 128] all < HALO
        pos_slot = p_pad[rows].transpose(1, 0, 2).reshape(128, NT * 3)
        ori_slot = o_pad[rows].transpose(1, 0, 2).reshape(128, NT * 9)
        # receiver gathers, tile-major: [m, (t, j, d)], row = m + j - 8
        mm = np.arange(128)[:, None, None]
        tt2 = np.arange(NT)[None, :, None]
        kk = np.arange(K17)[None, None, :]
        r = mm + kk - 8
        okr = (r >= 0) & (r < 136)
        gidx = TS * tt2 + np.clip(r, 0, 135)
        pos_gath = np.ascontiguousarray(
            np.where(okr[..., None], p_pad[gidx], 0.0)
            .reshape(128, NT * K17 * 3)).astype(np.float32)
        ori_gath_f = np.ascontiguousarray(
            np.where(okr[..., None], o_pad[gidx], 0.0)
            .reshape(128, NT * K17 * 9)).astype(np.float32)
        rc = WIN + TS * jj + pp
        okc = rc < HALO
        xc_slot = np.where(okc[:, :, None], x_pad[np.minimum(rc, HALO - 1)], 0.0)
        xc_slot = xc_slot.transpose(1, 0, 2).reshape(128, NT * C).astype(np.float32)

        # mask[m, t, j]: edge (n = m-16+j, offset 8-j) exists;
        # ncl[m, t]: #folded self-loops of node m-8
        mask = np.zeros((128, NT, K17), np.float32)
        ncl = np.zeros((128, NT), np.float32)
        for t in range(NT):
            cnt = min(TS, NPC - TS * t)
            for m in range(128):
                for j in range(K17):
                    n = m - 16 + j
                    if 0 <= n < cnt:
                        off = (s0 + TS * t + n) % L
                        if 0 <= off + 8 - j < L:
                            mask[m, t, j] = 1.0
                nn = m - WIN
                if 0 <= nn < cnt:
                    off = (s0 + TS * t + nn) % L
                    v = ((off + offs) >= 0) & ((off + offs) < L)
                    ncl[m, t] = K17 - v.sum()
        wbf = np.zeros((128, 1634), np.float32)
        wbf[:, 0:128] = np.eye(128)
        wbf[:, 128:298] = mask.reshape(128, NT * K17)
        wbf[0:NF, 298:706] = WS
        wbf[0:W, 706:1474] = wkoc
        wbf[0:W, 1474:1602] = np.ascontiguousarray(W_out)
        wbf[:, 1602:1634] = np.ascontiguousarray(W_in)
        wfc = wf.copy()
        wfc[:, 96:106] = ncl
        gpack_f = np.zeros((128, 540), np.float32)
        gpack_f[:, 0:30] = pos_slot
        gpack_f[:, 30:540] = pos_gath
        gpack_bf = np.zeros((128, 1620), np.float32)
        gpack_bf[:, 0:90] = ori_slot
        gpack_bf[:, 90:1620] = ori_gath_f
        in_maps.append(dict(
            xc_slot=xc_slot, gpack_f=gpack_f,
            xT_slot=np.ascontiguousarray(x_pad.T).astype(bf),
            gpack_bf=gpack_bf.astype(bf), wpack_bf=wbf.astype(bf),
            wpack_f=wfc))
    return in_maps


def kernel(x, pos, seq, ori, W_in, Ws0, bs0, Wk, W_out, src, dst):
    exp_src, exp_dst = _expected_src_dst()
    assert np.array_equal(np.asarray(src), exp_src), "unexpected src graph"
    assert np.array_equal(np.asarray(dst), exp_dst), "unexpected dst graph"

    from concourse.bass_utils import run_bass_kernel_spmd

    if "nc" not in _PROG:
        _PROG["nc"] = _build_program()
    nc = _PROG["nc"]

    in_maps = _host_inputs(np.asarray(x), np.asarray(pos), np.asarray(ori),
                           np.asarray(W_in), np.asarray(Ws0), np.asarray(bs0),
                           np.asarray(Wk), np.asarray(W_out))
    res = run_bass_kernel_spmd(nc, in_maps, list(range(NCORES)))
    out = np.concatenate([res.results[i]["y"] for i in range(NCORES)], axis=0)
    return out.reshape(B, L, C).astype(np.float32)
